# revision 3
# baseline (speedup 1.0000x reference)
"""Distributed Trainium2 kernel for the pairwise-distance alignment loss.

Math (per loss pair (x, y), scale s = 1/(tau*sqrt(D))):
    pos_i  = s*||x_i - y_i||
    dm_ij  = s*||x_i - y_j||
    loss   = mean_i( pos_i - log(sum_j exp(dm_ij)) )
computed for y = label_prompt_embedding (center) and y = aug_x (instance).

Distribution: shard the N=1024 rows of x across 8 NeuronCores (128 rows
each); every core holds the full y (replicated) and computes its
[128, 1024] block of each pairwise matrix, reducing rows locally.

Device algorithm (per core), using the Gram trick
    s^2*||x_i - y_j||^2 = s^2*xsq_i + s^2*ysq_j - 2*s^2*x_i.y_j:
  - main matmul: PSUM  = ((-2*s^2*x)^T)^T @ y^T          (K=128)
  - rank-1 matmul: PSUM += ones^T @ (s^2*ysq)            (K=1)
  - ACT pass 1: t = Ln(PSUM + bias),  bias_i = s^2*xsq_i + eps  (per-partition)
  - ACT pass 2: dm = Exp(0.5*t)       == sqrt of the biased square
  - ACT pass 3: E = Exp(dm), accum_out -> row sums (denom)
  - ACT pass 4: out = Ln(denom)
  sqrt is computed as exp(0.5*ln(v)) so the whole kernel needs a single
  ACT table set (natural_log_exp_and_others) -> one hidden table load.

Host prepares transposed/pre-scaled operands (O(N*D) work) and does the
O(N) epilogue: pos terms and the final means.
"""

import numpy as np

import concourse.bass as bass
import concourse.mybir as mybir
from concourse import bacc, tile
from concourse.bass_utils import run_bass_kernel_spmd

N, D, NCORES = 1024, 128, 8
ROWS = N // NCORES          # 128 rows of x per core
TAU, BETA = 1.0, 1.0
S2 = 1.0 / (TAU * TAU * D)  # scale^2
EPS = 1e-3                  # guards ln() against tiny negative Gram residuals
CH = 512                    # free-dim chunk (1 PSUM bank)
NCH = 2 * N // CH           # 4 chunks: 2 for center-y, 2 for instance-y

_NC_CACHE = None

# The greedy ACT table-set chooser picks the first set containing each
# activation's function, which thrashes between `natural_log` (Ln) and
# `exp_and_others` (Exp) -- ~1.3us per reload. Both functions live in
# `natural_log_exp_and_others`; steer the chooser there by hiding that
# set's functions from every other set (indices/ids stay untouched).
_COMBINED_SET = "natural_log_exp_and_others"


def _patched_get_activation_tables(arch):
    from concourse.hw_specs import get_activation_tables as _orig

    tabs = _orig(arch)
    target = tabs[_COMBINED_SET]
    return {
        name: (funcs if name == _COMBINED_SET else funcs - target)
        for name, funcs in tabs.items()
    }


def _build():
    f32 = mybir.dt.float32
    AF = mybir.ActivationFunctionType
    nc = bacc.Bacc("TRN2", target_bir_lowering=False, debug=False,
                   num_devices=NCORES)

    xT2_d = nc.dram_tensor("xT2", [D, ROWS], f32, kind="ExternalInput")
    yT_d = nc.dram_tensor("yT", [D, 2 * N], f32, kind="ExternalInput")
    ysq_d = nc.dram_tensor("ysq", [1, 2 * N], f32, kind="ExternalInput")
    bias_d = nc.dram_tensor("biasv", [ROWS, 1], f32, kind="ExternalInput")
    out_d = nc.dram_tensor("out", [ROWS, 2], f32, kind="ExternalOutput")

    with tile.TileContext(nc) as tc:
        with (
            tc.tile_pool(name="const", bufs=1) as cpool,
            tc.tile_pool(name="ybuf", bufs=NCH) as ypool,
            tc.tile_pool(name="work", bufs=2) as wpool,
            tc.tile_pool(name="psum", bufs=2, space="PSUM") as ppool,
            tc.tile_pool(name="stat", bufs=1) as spool,
        ):
            xt = cpool.tile([D, ROWS], f32)
            nc.sync.dma_start(xt[:], xT2_d[:])
            ysq = cpool.tile([1, 2 * N], f32)
            nc.sync.dma_start(ysq[:], ysq_d[:])
            biasv = cpool.tile([ROWS, 1], f32)
            nc.sync.dma_start(biasv[:], bias_d[:])
            ones = cpool.tile([1, ROWS], f32)
            nc.gpsimd.memset(ones[:], 1.0)

            den = spool.tile([ROWS, NCH], f32)
            den2 = spool.tile([ROWS, 2], f32)
            outt = spool.tile([ROWS, 2], f32)

            for c in range(NCH):
                sl = slice(c * CH, (c + 1) * CH)
                yc = ypool.tile([D, CH], f32)
                nc.sync.dma_start(yc[:], yT_d[:, sl])
                ps = ppool.tile([ROWS, CH], f32)
                nc.tensor.matmul(ps[:], xt[:], yc[:], start=True, stop=False)
                nc.tensor.matmul(ps[:], ones[:], ysq[:, sl], start=False,
                                 stop=True)
                t1 = wpool.tile([ROWS, CH], f32, tag="t1")
                nc.scalar.activation(t1[:], ps[:], AF.Ln, bias=biasv[:])
                t2 = wpool.tile([ROWS, CH], f32, tag="t2")
                nc.scalar.activation(t2[:], t1[:], AF.Exp, scale=0.5)
                t3 = wpool.tile([ROWS, CH], f32, tag="t3")
                nc.scalar.activation(t3[:], t2[:], AF.Exp,
                                     accum_out=den[:, c:c + 1])

            nc.vector.tensor_add(den2[:, 0:1], den[:, 0:1], den[:, 1:2])
            nc.vector.tensor_add(den2[:, 1:2], den[:, 2:3], den[:, 3:4])
            nc.scalar.activation(outt[:], den2[:], AF.Ln)
            nc.sync.dma_start(out_d[:], outt[:])

    _orig_tables = bacc.get_activation_tables
    bacc.get_activation_tables = _patched_get_activation_tables
    try:
        nc.compile()
    finally:
        bacc.get_activation_tables = _orig_tables
    return nc


def _get_nc():
    global _NC_CACHE
    if _NC_CACHE is None:
        _NC_CACHE = _build()
    return _NC_CACHE


def kernel(x, aug_x, label_prompt_embedding):
    x = np.asarray(x, dtype=np.float32)
    aug = np.asarray(aug_x, dtype=np.float32)
    lab = np.asarray(label_prompt_embedding, dtype=np.float32)

    s2 = np.float32(S2)
    # [D, N] pre-transposed, pre-scaled matmul operands
    xT2 = np.ascontiguousarray((x * (-2.0 * s2)).T.astype(np.float32))
    yT = np.ascontiguousarray(np.concatenate([lab, aug], axis=0).T
                              .astype(np.float32))          # [D, 2N]
    ysq = (s2 * np.concatenate([(lab * lab).sum(1), (aug * aug).sum(1)])
           ).astype(np.float32)[None, :]                    # [1, 2N]
    biasv = (s2 * (x * x).sum(1) + np.float32(EPS)).astype(np.float32)[:, None]

    in_maps = [
        {
            "xT2": np.ascontiguousarray(xT2[:, k * ROWS:(k + 1) * ROWS]),
            "yT": yT,
            "ysq": ysq,
            "biasv": biasv[k * ROWS:(k + 1) * ROWS],
        }
        for k in range(NCORES)
    ]

    nc = _get_nc()
    res = run_bass_kernel_spmd(nc, in_maps, list(range(NCORES))).results
    lnden = np.concatenate([res[k]["out"] for k in range(NCORES)], axis=0)

    # Host epilogue: positive-pair distances and final means (O(N*D)).
    s = np.float32(1.0 / (TAU * np.sqrt(np.float32(D))))
    pos_c = np.sqrt(((x - lab) ** 2).sum(1)) * s
    pos_i = np.sqrt(((x - aug) ** 2).sum(1)) * s
    center = np.float32((pos_c - lnden[:, 0]).mean())
    inst = np.float32((pos_i - lnden[:, 1]).mean())
    total = np.float32(center + np.float32(BETA) * inst)
    return (total, center, inst)


# revision 4
# speedup vs baseline: 1.3539x; 1.3539x over previous
"""Distributed Trainium2 kernel for the pairwise-distance alignment loss.

Math (per loss pair (x, y), scale s = 1/(tau*sqrt(D))):
    pos_i  = s*||x_i - y_i||
    dm_ij  = s*||x_i - y_j||
    loss   = mean_i( pos_i - log(sum_j exp(dm_ij)) )
computed for y = label_prompt_embedding (center) and y = aug_x (instance).

Distribution: shard the N=1024 rows of x across 8 NeuronCores (128 rows
each); every core holds the full y (replicated) and computes its
[128, 1024] block of each pairwise matrix, reducing rows locally.

Device algorithm (per core), using the Gram trick
    s^2*||x_i - y_j||^2 = s^2*xsq_i + s^2*ysq_j - 2*s^2*x_i.y_j:
  - main matmul (bf16): PSUM  = ((-2*s^2*x)^T)^T @ y^T    (K=128)
  - rank-1 matmul (bf16): PSUM += ones^T @ (s^2*ysq)      (K=1)
  - ACT pass 1: t = Ln(PSUM + bias), bias_i = s^2*xsq_i + eps (f32,
    per-partition)
  - ACT pass 2: dm = Exp(0.5*t)       == sqrt of the biased square
  - ACT pass 3: E = Exp(dm), accum_out -> row sums (denom) per y
  - ACT pass 4: out = Ln(denom)
  sqrt is computed as exp(0.5*ln(v)) so the whole kernel needs a single
  ACT table set (natural_log_exp_and_others) -> one hidden table load.

Host prepares transposed/pre-scaled bf16 operands (O(N*D) work) and does
the O(N) epilogue: pos terms and the final means.
"""

import numpy as np
import ml_dtypes

import concourse.bass as bass
import concourse.mybir as mybir
from concourse import bacc, tile
from concourse.bass_utils import run_bass_kernel_spmd

BF16 = ml_dtypes.bfloat16

N, D, NCORES = 1024, 128, 8
ROWS = N // NCORES          # 128 rows of x per core
TAU, BETA = 1.0, 1.0
S2 = 1.0 / (TAU * TAU * D)  # scale^2
EPS = 1e-3                  # guards ln() against tiny negative Gram residuals
CH = 1024                   # ACT pass width = one full y block (2 PSUM banks)
NCH = 2 * N // CH           # 2 chunks: center-y, instance-y

_NC_CACHE = None

# The greedy ACT table-set chooser picks the first set containing each
# activation's function, which thrashes between `natural_log` (Ln) and
# `exp_and_others` (Exp) -- ~1.3us per reload. Both functions live in
# `natural_log_exp_and_others`; steer the chooser there by hiding that
# set's functions from every other set (indices/ids stay untouched).
_COMBINED_SET = "natural_log_exp_and_others"


def _patched_get_activation_tables(arch):
    from concourse.hw_specs import get_activation_tables as _orig

    tabs = _orig(arch)
    target = tabs[_COMBINED_SET]
    return {
        name: (funcs if name == _COMBINED_SET else funcs - target)
        for name, funcs in tabs.items()
    }


def _build():
    f32 = mybir.dt.float32
    bf16 = mybir.dt.bfloat16
    AF = mybir.ActivationFunctionType
    nc = bacc.Bacc("TRN2", target_bir_lowering=False, debug=False,
                   num_devices=NCORES)

    xT2_d = nc.dram_tensor("xT2", [D, ROWS], bf16, kind="ExternalInput")
    yT_d = nc.dram_tensor("yT", [D, 2 * N], bf16, kind="ExternalInput")
    # ysq row + a trailing [1, ROWS] of ones (lhsT for the rank-1 matmul)
    ysq_d = nc.dram_tensor("ysq", [1, 2 * N + ROWS], bf16,
                           kind="ExternalInput")
    bias_d = nc.dram_tensor("biasv", [ROWS, 1], f32, kind="ExternalInput")
    out_d = nc.dram_tensor("out", [ROWS, 2], f32, kind="ExternalOutput")

    with tile.TileContext(nc) as tc:
        with (
            tc.tile_pool(name="const", bufs=1) as cpool,
            tc.tile_pool(name="ybuf", bufs=NCH) as ypool,
            tc.tile_pool(name="work", bufs=2) as wpool,
            tc.tile_pool(name="psum", bufs=NCH, space="PSUM") as ppool,
            tc.tile_pool(name="stat", bufs=1) as spool,
        ):
            xt = cpool.tile([D, ROWS], bf16)
            nc.sync.dma_start(xt[:], xT2_d[:])
            ysq = cpool.tile([1, 2 * N + ROWS], bf16)
            nc.scalar.dma_start(ysq[:], ysq_d[:])
            biasv = cpool.tile([ROWS, 1], f32)
            nc.scalar.dma_start(biasv[:], bias_d[:])
            ones = ysq[:, 2 * N:2 * N + ROWS]

            den = spool.tile([ROWS, NCH], f32)
            outt = spool.tile([ROWS, 2], f32)

            for c in range(NCH):
                yc = ypool.tile([D, CH], bf16)
                eng = nc.sync if c % 2 == 0 else nc.scalar
                eng.dma_start(yc[:], yT_d[:, c * CH:(c + 1) * CH])
                ps = ppool.tile([ROWS, CH], f32)
                for h in range(CH // 512):
                    hs = slice(h * 512, (h + 1) * 512)
                    gs = slice(c * CH + h * 512, c * CH + (h + 1) * 512)
                    nc.tensor.matmul(ps[:, hs], xt[:], yc[:, hs],
                                     start=True, stop=False)
                    nc.tensor.matmul(ps[:, hs], ones, ysq[:, gs],
                                     start=False, stop=True)
                t1 = wpool.tile([ROWS, CH], f32, tag="t1")
                nc.scalar.activation(t1[:], ps[:], AF.Ln, bias=biasv[:])
                t2 = wpool.tile([ROWS, CH], f32, tag="t2")
                nc.scalar.activation(t2[:], t1[:], AF.Exp, scale=0.5)
                t3 = wpool.tile([ROWS, CH], f32, tag="t3")
                nc.scalar.activation(t3[:], t2[:], AF.Exp,
                                     accum_out=den[:, c:c + 1])

            nc.scalar.activation(outt[:], den[:], AF.Ln)
            nc.sync.dma_start(out_d[:], outt[:])

    _orig_tables = bacc.get_activation_tables
    bacc.get_activation_tables = _patched_get_activation_tables
    try:
        nc.compile()
    finally:
        bacc.get_activation_tables = _orig_tables
    return nc


def _get_nc():
    global _NC_CACHE
    if _NC_CACHE is None:
        _NC_CACHE = _build()
    return _NC_CACHE


def _prep_in_maps(x, aug, lab):
    s2 = np.float32(S2)
    # [D, N] pre-transposed, pre-scaled matmul operands (bf16)
    xT2 = np.ascontiguousarray((x * (-2.0 * s2)).T).astype(BF16)
    yT = np.ascontiguousarray(np.concatenate([lab, aug], axis=0).T
                              ).astype(BF16)                # [D, 2N]
    ysq_row = (s2 * np.concatenate([(lab * lab).sum(1), (aug * aug).sum(1)])
               ).astype(BF16)
    ysq = np.concatenate([ysq_row, np.ones(ROWS, BF16)])[None, :]
    biasv = (s2 * (x * x).sum(1) + np.float32(EPS)).astype(np.float32)[:, None]

    return [
        {
            "xT2": np.ascontiguousarray(xT2[:, k * ROWS:(k + 1) * ROWS]),
            "yT": yT,
            "ysq": ysq,
            "biasv": biasv[k * ROWS:(k + 1) * ROWS],
        }
        for k in range(NCORES)
    ]


def kernel(x, aug_x, label_prompt_embedding):
    x = np.asarray(x, dtype=np.float32)
    aug = np.asarray(aug_x, dtype=np.float32)
    lab = np.asarray(label_prompt_embedding, dtype=np.float32)

    in_maps = _prep_in_maps(x, aug, lab)
    nc = _get_nc()
    res = run_bass_kernel_spmd(nc, in_maps, list(range(NCORES))).results
    lnden = np.concatenate([res[k]["out"] for k in range(NCORES)], axis=0)

    # Host epilogue: positive-pair distances and final means (O(N*D)).
    s = np.float32(1.0 / (TAU * np.sqrt(np.float32(D))))
    pos_c = np.sqrt(((x - lab) ** 2).sum(1)) * s
    pos_i = np.sqrt(((x - aug) ** 2).sum(1)) * s
    center = np.float32((pos_c - lnden[:, 0]).mean())
    inst = np.float32((pos_i - lnden[:, 1]).mean())
    total = np.float32(center + np.float32(BETA) * inst)
    return (total, center, inst)
